# revision 12
# baseline (speedup 1.0000x reference)
"""Trainium2 Bass kernel for Clique2NodeConvBasic (GNN message passing).

Computes, for N=100000 nodes, C=50000 cliques, E=1600000 edges, D=128:

    gathered = x_clique[clique_idx]            # [E, 128]
    summed   = segment_sum(gathered, node_idx) # [N, 128]
    mean     = summed / max(count, 1)
    out      = mean @ W.T + b                  # [N, 128]

Sharding: edges partitioned by destination-node range across 8 NeuronCores
(12500 nodes per core); x_clique and the Linear weights replicated.

v2 design (from microbenchmarks on this hardware):
  - The bottleneck is GpSimd Q7 descriptor generation inside dma_gather
    (~7.9 ns per gathered row on one SWDGE queue pair). dma_gather
    instructions issued on DIFFERENT SWDGE queues (queue_num 0-3) execute
    on different Q7 core pairs and OVERLAP: 4-queue round-robin measured
    2.53 ns/row effective (3.2x).
  - All float data is bf16: halves the DMA drain (256B descriptors) and
    doubles PE matmul throughput. rel err ~1e-3, gate is 2e-2.
  - Tight packing: per (core, block) tile counts are data-dependent; the
    SPMD program is uniform across cores by padding each block position to
    the max tile count over the 8 cores (~+5% rows vs ~+12% for the old
    global-max padding).
  - Per-block accumulate in PSUM via one-hot matmuls (gathered tile is the
    STATIONARY operand; the moving-operand path crashes on dma_gather-
    written tiles), then Linear + 1/count scale + bias epilogue per block.
  - ap_gather / scatter_add / trailing -1 trimming / single_packet=True
    were all benched: ap_gather is 27.7 ns/idx, -1 trimming makes calls
    slower, single_packet=True hangs the device. Avoided.
"""

import os
import sys
import types

sys.path.insert(0, "/opt/trn_rl_repo")

import numpy as np

import concourse.bass as bass
import concourse.mybir as mybir
import concourse.tile as tile
from concourse.vector_clock import ScopedClock, VectorClock
from concourse.bass_utils import run_bass_kernel_spmd

# ----------------------------------------------------------------------------
# Environment shims
# ----------------------------------------------------------------------------

def _install_ntff_shim():
    """Register the axon NTFF profile hook if the image's antenv lacks it."""
    try:
        import antenv
    except ImportError:
        return
    if hasattr(antenv, "axon_hooks"):
        return
    hooks_mod = types.ModuleType("antenv.axon_hooks")
    _store = [None]
    hooks_mod.set_axon_ntff_profile_hook = lambda h: _store.__setitem__(0, h)
    hooks_mod.get_axon_ntff_profile_hook = lambda: _store[0]
    sys.modules["antenv.axon_hooks"] = hooks_mod
    antenv.axon_hooks = hooks_mod
    try:
        from trn_agent_boot.trn_boot import _ntff_profile_via_ctypes

        hook = _ntff_profile_via_ctypes("/opt/axon/libaxon_pjrt.so")
        if hook is not None:
            hooks_mod.set_axon_ntff_profile_hook(hook)
    except Exception:
        pass


_install_ntff_shim()


class PatchedTileContext(tile.TileContext):
    """Spread the tail-drain's sem waits over a chain of SP NOPs.

    The walrus build in this container caps sync-waits per instruction
    (setupSyncWait: "Too many sync wait commands"), while stock Tile
    attaches every outstanding proc's wait to one Drain. One NOP per
    proc keeps every instruction at a single wait.
    """

    def _drain_and_barrier(self, tick_clock, wait_clock):
        gc = tick_clock.global_clock
        for p, t in enumerate(gc):
            if t <= 0:
                continue
            nop = self.nc.sync.nop()
            part = VectorClock()
            part.require_at_least(p, t)
            wait_clock.add_sem_waits(nop.ins, ScopedClock({None: part}))
        self.nc.sync.drain()
        self.nc.all_engine_barrier()
        assert self.sems is not None
        popped = self.nc._tile_sem_poison_stack.pop()
        assert popped is self._sem_poison
        self.nc.clear_and_free_semaphores(list(self.sems.allocated().values()))
        self.nc.all_engine_barrier()


# ----------------------------------------------------------------------------
# Problem constants (hardcoded per the task contract)
# ----------------------------------------------------------------------------

N_NODES = 100000
N_CLIQUES = 50000
D = 128
N_CORES = 8
NPC = N_NODES // N_CORES        # 12500 nodes per core
BLK = 128                       # destination nodes per block
NBLK = -(-NPC // BLK)           # 98 blocks per core (last partial: 84)
NPAD = NBLK * BLK               # 12544 padded output rows per core
SPLIT = 32768                   # int16-index limit for dma_gather
NT = 24                         # 128-row tiles per dma_gather call
NQ = 4                          # SWDGE queues used round-robin
PAD_DEST = -1000.0              # one-hot miss value for padding slots

_F32 = mybir.dt.float32
_BF16 = mybir.dt.bfloat16
_FP8 = mybir.dt.float8e4

import ml_dtypes

_NP_BF16 = np.dtype(ml_dtypes.bfloat16)


# ----------------------------------------------------------------------------
# Host-side preparation
# ----------------------------------------------------------------------------

def _prepare(x_clique, node2clique_index):
    """Sort/bucket the edge list. Returns per-core input dicts plus the
    (data-dependent) uniform schedule."""
    node = np.asarray(node2clique_index[0]).astype(np.int64)
    clique = np.asarray(node2clique_index[1]).astype(np.int64)

    counts = np.bincount(node, minlength=N_NODES).astype(np.float64)
    inv_cnt = (1.0 / np.maximum(counts, 1.0)).astype(np.float32)

    order = np.argsort(node, kind="stable")
    ns = node[order]
    cs = clique[order]
    core_bounds = np.searchsorted(ns, np.arange(N_CORES + 1) * NPC)

    # Per-core stable partition: (block, is_b) groups, A before B.
    per_core = []
    cntA = np.zeros((N_CORES, NBLK), dtype=np.int64)
    cntB = np.zeros((N_CORES, NBLK), dtype=np.int64)
    for c in range(N_CORES):
        lo, hi = core_bounds[c], core_bounds[c + 1]
        loc = ns[lo:hi] - c * NPC
        cq = cs[lo:hi]
        blk = loc // BLK
        win = loc % BLK
        is_b = cq >= SPLIT
        key = blk * 2 + is_b
        sub = np.argsort(key, kind="stable")
        blk, win, cq, is_b = blk[sub], win[sub], cq[sub], is_b[sub]
        cntA[c] = np.bincount(blk[~is_b], minlength=NBLK)
        cntB[c] = np.bincount(blk[is_b], minlength=NBLK)
        per_core.append((blk, win, cq, is_b))

    # Uniform schedule: per block position, max tile count over cores.
    tAmax = -(-cntA.max(axis=0) // 128)           # [NBLK]
    tBmax = np.maximum(-(-cntB.max(axis=0) // 128), 1)
    offA = np.concatenate([[0], np.cumsum(tAmax)])  # tile offset per block
    offB = np.concatenate([[0], np.cumsum(tBmax)])
    totA, totB = int(offA[-1]), int(offB[-1])
    callsA, callsB = -(-totA // NT), -(-totB // NT)
    padA, padB = callsA * NT, callsB * NT

    def _wrap(idx):
        # [L] -> [128, L/16] (16-partition wrap, replicated to 8 core groups)
        w = idx.reshape(-1, 16).T.copy().reshape(16, -1)
        return np.tile(w, (8, 1))

    in_maps = []
    for c in range(N_CORES):
        blk, win, cq, is_b = per_core[c]
        idxA = np.zeros(padA * 128, dtype=np.int16)
        idxB = np.zeros(padB * 128, dtype=np.int16)
        destA = np.full(padA * 128, PAD_DEST, dtype=np.float32)
        destB = np.full(padB * 128, PAD_DEST, dtype=np.float32)

        a = ~is_b
        posA = np.arange(a.sum()) - np.concatenate([[0], np.cumsum(cntA[c])])[blk[a]]
        posB = np.arange(is_b.sum()) - np.concatenate([[0], np.cumsum(cntB[c])])[blk[is_b]]
        slotA = offA[blk[a]] * 128 + posA
        slotB = offB[blk[is_b]] * 128 + posB
        idxA[slotA] = cq[a].astype(np.int16)
        idxB[slotB] = (cq[is_b] - SPLIT).astype(np.int16)
        destA[slotA] = win[a]
        destB[slotB] = win[is_b]

        # dest layout for is_equal: [128 slot, n_tiles]
        destA_t = np.ascontiguousarray(destA.reshape(padA, 128).T).astype(_NP_BF16)
        destB_t = np.ascontiguousarray(destB.reshape(padB, 128).T).astype(_NP_BF16)

        inv_t = np.zeros((BLK, NBLK), dtype=np.float32)
        inv_t.T.flat[:NPC] = inv_cnt[c * NPC : (c + 1) * NPC]

        # max(cnt,1) so zero-count nodes still get +bias after the 1/max(cnt,1)
        # scale: (0 + 1*b)*1 = b, matching segment-mean-with-clamp semantics.
        cnt_row = np.zeros((1, NPAD), dtype=np.float32)
        cnt_row[0, :NPC] = np.maximum(counts[c * NPC : (c + 1) * NPC], 1.0)

        in_maps.append(
            {
                "idxA": _wrap(idxA),
                "idxB": _wrap(idxB),
                "destA": destA_t,
                "destB": destB_t,
                "invc": np.ascontiguousarray(inv_t),
                "cntb": cnt_row.astype(_NP_BF16),
            }
        )

    xc = np.asarray(x_clique)
    shared = {
        "xcA": np.ascontiguousarray(xc[:SPLIT]).astype(_NP_BF16),
        "xcB": np.ascontiguousarray(xc[SPLIT:]).astype(_NP_BF16),
        "iota": np.tile(np.arange(128, dtype=np.float32), (128, 1)).astype(_NP_BF16),
    }
    sched = (tuple(int(t) for t in tAmax), tuple(int(t) for t in tBmax))
    return in_maps, shared, sched


# ----------------------------------------------------------------------------
# Kernel builder
# ----------------------------------------------------------------------------

def _build(sched):
    tAmax, tBmax = np.array(sched[0]), np.array(sched[1])
    offA = np.concatenate([[0], np.cumsum(tAmax)])
    offB = np.concatenate([[0], np.cumsum(tBmax)])
    totA, totB = int(offA[-1]), int(offB[-1])
    callsA, callsB = -(-totA // NT), -(-totB // NT)
    padA, padB = callsA * NT, callsB * NT
    CB = N_CLIQUES - SPLIT

    from concourse.bacc import Bacc

    nc = Bacc(None, num_swdge_queues=NQ)
    xcA = nc.declare_dram_parameter("xcA", [SPLIT, D], _BF16, isOutput=False)
    xcB = nc.declare_dram_parameter("xcB", [CB, D], _BF16, isOutput=False)
    idxA = nc.declare_dram_parameter("idxA", [128, padA * 8], mybir.dt.int16, isOutput=False)
    idxB = nc.declare_dram_parameter("idxB", [128, padB * 8], mybir.dt.int16, isOutput=False)
    destA = nc.declare_dram_parameter("destA", [128, padA], _BF16, isOutput=False)
    destB = nc.declare_dram_parameter("destB", [128, padB], _BF16, isOutput=False)
    invc = nc.declare_dram_parameter("invc", [128, NBLK], _F32, isOutput=False)
    iota = nc.declare_dram_parameter("iota", [128, 128], _BF16, isOutput=False)
    wt = nc.declare_dram_parameter("wt", [128, 128], _BF16, isOutput=False)
    brow = nc.declare_dram_parameter("brow", [1, 128], _BF16, isOutput=False)
    cntb = nc.declare_dram_parameter("cntb", [1, NPAD], _BF16, isOutput=False)
    out = nc.declare_dram_parameter("out", [NPAD, D], _F32, isOutput=True)

    # Gather-call lists per stream: full NT-tile calls, but the final call is
    # split into <=8-tile sub-calls so the tail blocks' data lands sooner.
    def call_list(tot):
        calls = []
        t = 0
        while t < tot:
            nt = NT if tot - t > NT else min(8, tot - t)
            calls.append((t, nt))
            t += nt
        return calls

    callsA_l = call_list(totA)
    callsB_l = call_list(totB)

    # merged gather-call order: by first block each call serves (A first on tie)
    def start_block(off, t0):
        return int(np.searchsorted(off, t0, side="right") - 1)

    merged = sorted(
        [(start_block(offA, t0), 0, i) for i, (t0, nt) in enumerate(callsA_l)]
        + [(start_block(offB, t0), 1, i) for i, (t0, nt) in enumerate(callsB_l)],
        key=lambda t: (t[0], t[1], t[2]),
    )

    # tile index -> (call index, slot) per stream
    def tile_map(calls):
        m = {}
        for i, (t0, nt) in enumerate(calls):
            for s in range(nt):
                m[t0 + s] = (i, s)
        return m

    tmapA = tile_map(callsA_l)
    tmapB = tile_map(callsB_l)

    from contextlib import ExitStack

    with PatchedTileContext(nc) as tc, ExitStack() as ctx:
        const = ctx.enter_context(tc.tile_pool(name="const", bufs=1))
        gpool = ctx.enter_context(tc.tile_pool(name="g", bufs=3))
        opool = ctx.enter_context(tc.tile_pool(name="o", bufs=3))
        sb = ctx.enter_context(tc.tile_pool(name="sb", bufs=2))
        ps = ctx.enter_context(tc.tile_pool(name="ps", bufs=4, space="PSUM"))
        psl = ctx.enter_context(tc.tile_pool(name="psl", bufs=2, space="PSUM"))

        # idx consts loaded in chunks so the first gathers wait only on the
        # slices they read, not the whole index upload.
        idxA_t = const.tile([128, padA * 8], mybir.dt.int16)
        h1 = (padA * 8) // 4
        nc.sync.dma_start(idxA_t[:, :h1], idxA[:, :h1])
        nc.sync.dma_start(idxA_t[:, h1:], idxA[:, h1:])
        idxB_t = const.tile([128, padB * 8], mybir.dt.int16)
        h2 = (padB * 8) // 4
        nc.sync.dma_start(idxB_t[:, :h2], idxB[:, :h2])
        nc.sync.dma_start(idxB_t[:, h2:], idxB[:, h2:])
        destA_t = const.tile([128, padA], _BF16)
        nc.sync.dma_start(destA_t[:], destA[:])
        destB_t = const.tile([128, padB], _BF16)
        nc.sync.dma_start(destB_t[:], destB[:])
        invc_t = const.tile([128, NBLK], _F32)
        nc.sync.dma_start(invc_t[:], invc[:])
        iota_t = const.tile([128, 128], _BF16)
        nc.sync.dma_start(iota_t[:], iota[:])
        wt_t = const.tile([128, 128], _BF16)
        nc.sync.dma_start(wt_t[:], wt[:])
        brow_t = const.tile([1, 128], _BF16)
        nc.sync.dma_start(brow_t[:], brow[:])
        cntb_t = const.tile([1, NPAD], _BF16)
        nc.sync.dma_start(cntb_t[:], cntb[:])

        call_tiles = {}   # (stream, k) -> (gathered tile, onehot tile)
        emitted = [0]

        def emit_calls(up_to_block):
            while emitted[0] < len(merged) and merged[emitted[0]][0] <= up_to_block:
                _, stream, k = merged[emitted[0]]
                qi = emitted[0] % NQ
                src = xcA if stream == 0 else xcB
                idx_t = idxA_t if stream == 0 else idxB_t
                dest_t = destA_t if stream == 0 else destB_t
                t0, nt = (callsA_l if stream == 0 else callsB_l)[k]
                g_t = gpool.tile([128, NT, 128], _BF16, tag=f"g{qi}")
                nc.gpsimd.dma_gather(
                    g_t[:, :nt, :],
                    src[:],
                    idx_t[:, t0 * 8 : (t0 + nt) * 8],
                    nt * 128,
                    nt * 128,
                    D,
                    single_packet=False,
                    queue_num=qi,
                )
                oh_t = opool.tile([128, NT, 128], _FP8, tag=f"o{qi}")
                nc.vector.tensor_tensor(
                    out=oh_t[:, :nt, :],
                    in0=dest_t[:, t0 : t0 + nt, None].to_broadcast(
                        [128, nt, 128]
                    ),
                    in1=iota_t[:, None, :].to_broadcast([128, nt, 128]),
                    op=mybir.AluOpType.is_equal,
                )
                call_tiles[(stream, k)] = (g_t, oh_t)
                emitted[0] += 1

        for b in range(NBLK):
            emit_calls(b)
            mms = [(0, t) for t in range(int(offA[b]), int(offA[b + 1]))] + [
                (1, t) for t in range(int(offB[b]), int(offB[b + 1]))
            ]
            accum = ps.tile([128, 128], _F32, tag="acc")
            for i, (stream, t) in enumerate(mms):
                ci, slot = (tmapA if stream == 0 else tmapB)[t]
                g_t, oh_t = call_tiles[(stream, ci)]
                nc.tensor.matmul(
                    out=accum[:],
                    lhsT=g_t[:, slot, :],
                    rhs=oh_t[:, slot, :],
                    start=(i == 0),
                    stop=(i == len(mms) - 1),
                )
            # accum[f, n] is summed.T — exactly the lhsT the Linear wants.
            acc_sb = sb.tile([128, 128], _BF16, tag="accsb")
            nc.scalar.activation(
                acc_sb[:], accum[:], mybir.ActivationFunctionType.Copy
            )
            # lin[n, o] = summed[n, :] @ W.T + max(cnt[n],1)*b[o]; the rank-1
            # count*bias term makes the later 1/max(cnt,1) scale yield "+b".
            lin = psl.tile([128, 128], _F32, tag="lin")
            nc.tensor.matmul(
                out=lin[:], lhsT=acc_sb[:], rhs=wt_t[:], start=True, stop=False
            )
            nc.tensor.matmul(
                out=lin[:],
                lhsT=cntb_t[:, b * 128 : (b + 1) * 128],
                rhs=brow_t[:],
                start=False,
                stop=True,
            )
            # out[n, o] = lin[n, o] / max(count[n], 1)
            sc = sb.tile([128, 128], _F32, tag="sc")
            nc.scalar.activation(
                sc[:],
                lin[:],
                mybir.ActivationFunctionType.Copy,
                scale=invc_t[:, b : b + 1],
            )
            nc.sync.dma_start(out[b * 128 : (b + 1) * 128, :], sc[:])

    nc.finalize()
    return nc


_BUILD_CACHE = {}


def kernel(x, x_clique, node2clique_index, W, b, _trace=False, _tmpdir=None):
    in_maps, shared, sched = _prepare(x_clique, node2clique_index)

    shared["wt"] = np.ascontiguousarray(
        np.asarray(W, dtype=np.float32).T
    ).astype(_NP_BF16)
    shared["brow"] = np.asarray(b, dtype=np.float32)[None, :].astype(_NP_BF16)

    if sched not in _BUILD_CACHE:
        _BUILD_CACHE[sched] = _build(sched)
    nc = _BUILD_CACHE[sched]

    full_maps = [dict(m, **shared) for m in in_maps]
    kwargs = {}
    if _trace:
        kwargs = dict(trace=True, tmpdir=_tmpdir)
    res = run_bass_kernel_spmd(nc, full_maps, core_ids=list(range(N_CORES)), **kwargs)

    out = np.concatenate(
        [res.results[c]["out"][:NPC] for c in range(N_CORES)], axis=0
    ).astype(np.float32)
    if _trace:
        return out, res
    return out


# revision 13
# speedup vs baseline: 1.1518x; 1.1518x over previous
"""Trainium2 Bass kernel for Clique2NodeConvBasic (GNN message passing).

Computes, for N=100000 nodes, C=50000 cliques, E=1600000 edges, D=128:

    gathered = x_clique[clique_idx]            # [E, 128]
    summed   = segment_sum(gathered, node_idx) # [N, 128]
    mean     = summed / max(count, 1)
    out      = mean @ W.T + b                  # [N, 128]

Sharding: edges partitioned by destination-node range across 8 NeuronCores
(12500 nodes per core); x_clique and the Linear weights replicated.

v2 design (from microbenchmarks on this hardware):
  - The bottleneck is GpSimd Q7 descriptor generation inside dma_gather
    (~7.9 ns per gathered row on one SWDGE queue pair). dma_gather
    instructions issued on DIFFERENT SWDGE queues (queue_num 0-3) execute
    on different Q7 core pairs and OVERLAP: 4-queue round-robin measured
    2.53 ns/row effective (3.2x).
  - All float data is bf16: halves the DMA drain (256B descriptors) and
    doubles PE matmul throughput. rel err ~1e-3, gate is 2e-2.
  - Tight packing: per (core, block) tile counts are data-dependent; the
    SPMD program is uniform across cores by padding each block position to
    the max tile count over the 8 cores (~+5% rows vs ~+12% for the old
    global-max padding).
  - Per-block accumulate in PSUM via one-hot matmuls (gathered tile is the
    STATIONARY operand; the moving-operand path crashes on dma_gather-
    written tiles), then Linear + 1/count scale + bias epilogue per block.
  - ap_gather / scatter_add / trailing -1 trimming / single_packet=True
    were all benched: ap_gather is 27.7 ns/idx, -1 trimming makes calls
    slower, single_packet=True hangs the device. Avoided.
"""

import os
import sys
import types

sys.path.insert(0, "/opt/trn_rl_repo")

import numpy as np

import concourse.bass as bass
import concourse.mybir as mybir
import concourse.tile as tile
from concourse.vector_clock import ScopedClock, VectorClock
from concourse.bass_utils import run_bass_kernel_spmd

# ----------------------------------------------------------------------------
# Environment shims
# ----------------------------------------------------------------------------

def _install_ntff_shim():
    """Register the axon NTFF profile hook if the image's antenv lacks it."""
    try:
        import antenv
    except ImportError:
        return
    if hasattr(antenv, "axon_hooks"):
        return
    hooks_mod = types.ModuleType("antenv.axon_hooks")
    _store = [None]
    hooks_mod.set_axon_ntff_profile_hook = lambda h: _store.__setitem__(0, h)
    hooks_mod.get_axon_ntff_profile_hook = lambda: _store[0]
    sys.modules["antenv.axon_hooks"] = hooks_mod
    antenv.axon_hooks = hooks_mod
    try:
        from trn_agent_boot.trn_boot import _ntff_profile_via_ctypes

        hook = _ntff_profile_via_ctypes("/opt/axon/libaxon_pjrt.so")
        if hook is not None:
            hooks_mod.set_axon_ntff_profile_hook(hook)
    except Exception:
        pass


_install_ntff_shim()


class PatchedTileContext(tile.TileContext):
    """Spread the tail-drain's sem waits over a chain of SP NOPs.

    The walrus build in this container caps sync-waits per instruction
    (setupSyncWait: "Too many sync wait commands"), while stock Tile
    attaches every outstanding proc's wait to one Drain. One NOP per
    proc keeps every instruction at a single wait.
    """

    def _drain_and_barrier(self, tick_clock, wait_clock):
        gc = tick_clock.global_clock
        for p, t in enumerate(gc):
            if t <= 0:
                continue
            nop = self.nc.sync.nop()
            part = VectorClock()
            part.require_at_least(p, t)
            wait_clock.add_sem_waits(nop.ins, ScopedClock({None: part}))
        self.nc.sync.drain()
        self.nc.all_engine_barrier()
        assert self.sems is not None
        popped = self.nc._tile_sem_poison_stack.pop()
        assert popped is self._sem_poison
        self.nc.clear_and_free_semaphores(list(self.sems.allocated().values()))
        self.nc.all_engine_barrier()


# ----------------------------------------------------------------------------
# Problem constants (hardcoded per the task contract)
# ----------------------------------------------------------------------------

N_NODES = 100000
N_CLIQUES = 50000
D = 128
N_CORES = 8
NPC = N_NODES // N_CORES        # 12500 nodes per core
BLK = 128                       # destination nodes per block
NBLK = -(-NPC // BLK)           # 98 blocks per core (last partial: 84)
NPAD = NBLK * BLK               # 12544 padded output rows per core
SPLIT = 32768                   # int16-index limit for dma_gather
NT = 24                         # 128-row tiles per dma_gather call
NQ = 4                          # SWDGE queues used round-robin
PAD_DEST = -1000.0              # one-hot miss value for padding slots

_F32 = mybir.dt.float32
_BF16 = mybir.dt.bfloat16
_FP8 = mybir.dt.float8e4

import ml_dtypes

_NP_BF16 = np.dtype(ml_dtypes.bfloat16)


# ----------------------------------------------------------------------------
# Host-side preparation
# ----------------------------------------------------------------------------

def _prepare(x_clique, node2clique_index):
    """Sort/bucket the edge list. Returns per-core input dicts plus the
    (data-dependent) uniform schedule."""
    node = np.asarray(node2clique_index[0]).astype(np.int64)
    clique = np.asarray(node2clique_index[1]).astype(np.int64)

    counts = np.bincount(node, minlength=N_NODES).astype(np.float64)
    inv_cnt = (1.0 / np.maximum(counts, 1.0)).astype(np.float32)

    order = np.argsort(node, kind="stable")
    ns = node[order]
    cs = clique[order]
    core_bounds = np.searchsorted(ns, np.arange(N_CORES + 1) * NPC)

    # Per-core stable partition: (block, is_b) groups, A before B.
    per_core = []
    cntA = np.zeros((N_CORES, NBLK), dtype=np.int64)
    cntB = np.zeros((N_CORES, NBLK), dtype=np.int64)
    for c in range(N_CORES):
        lo, hi = core_bounds[c], core_bounds[c + 1]
        loc = ns[lo:hi] - c * NPC
        cq = cs[lo:hi]
        blk = loc // BLK
        win = loc % BLK
        is_b = cq >= SPLIT
        key = blk * 2 + is_b
        sub = np.argsort(key, kind="stable")
        blk, win, cq, is_b = blk[sub], win[sub], cq[sub], is_b[sub]
        cntA[c] = np.bincount(blk[~is_b], minlength=NBLK)
        cntB[c] = np.bincount(blk[is_b], minlength=NBLK)
        per_core.append((blk, win, cq, is_b))

    # Uniform schedule: per block position, max tile count over cores.
    tAmax = -(-cntA.max(axis=0) // 128)           # [NBLK]
    tBmax = np.maximum(-(-cntB.max(axis=0) // 128), 1)
    offA = np.concatenate([[0], np.cumsum(tAmax)])  # tile offset per block
    offB = np.concatenate([[0], np.cumsum(tBmax)])
    totA, totB = int(offA[-1]), int(offB[-1])
    callsA, callsB = -(-totA // NT), -(-totB // NT)
    padA, padB = callsA * NT, callsB * NT

    def _wrap(idx):
        # [L] -> [128, L/16] (16-partition wrap, replicated to 8 core groups)
        w = idx.reshape(-1, 16).T.copy().reshape(16, -1)
        return np.tile(w, (8, 1))

    in_maps = []
    for c in range(N_CORES):
        blk, win, cq, is_b = per_core[c]
        idxA = np.zeros(padA * 128, dtype=np.int16)
        idxB = np.zeros(padB * 128, dtype=np.int16)
        destA = np.full(padA * 128, PAD_DEST, dtype=np.float32)
        destB = np.full(padB * 128, PAD_DEST, dtype=np.float32)

        a = ~is_b
        posA = np.arange(a.sum()) - np.concatenate([[0], np.cumsum(cntA[c])])[blk[a]]
        posB = np.arange(is_b.sum()) - np.concatenate([[0], np.cumsum(cntB[c])])[blk[is_b]]
        slotA = offA[blk[a]] * 128 + posA
        slotB = offB[blk[is_b]] * 128 + posB
        idxA[slotA] = cq[a].astype(np.int16)
        idxB[slotB] = (cq[is_b] - SPLIT).astype(np.int16)
        destA[slotA] = win[a]
        destB[slotB] = win[is_b]

        # dest layout for is_equal: [128 slot, n_tiles]
        destA_t = np.ascontiguousarray(destA.reshape(padA, 128).T).astype(_NP_BF16)
        destB_t = np.ascontiguousarray(destB.reshape(padB, 128).T).astype(_NP_BF16)

        inv_t = np.zeros((BLK, NBLK), dtype=np.float32)
        inv_t.T.flat[:NPC] = inv_cnt[c * NPC : (c + 1) * NPC]

        # max(cnt,1) so zero-count nodes still get +bias after the 1/max(cnt,1)
        # scale: (0 + 1*b)*1 = b, matching segment-mean-with-clamp semantics.
        cnt_row = np.zeros((1, NPAD), dtype=np.float32)
        cnt_row[0, :NPC] = np.maximum(counts[c * NPC : (c + 1) * NPC], 1.0)

        in_maps.append(
            {
                "idxA": _wrap(idxA),
                "idxB": _wrap(idxB),
                "destA": destA_t,
                "destB": destB_t,
                "invc": np.ascontiguousarray(inv_t),
                "cntb": cnt_row.astype(_NP_BF16),
            }
        )

    xc = np.asarray(x_clique)
    shared = {
        "xcA": np.ascontiguousarray(xc[:SPLIT]).astype(_NP_BF16),
        "xcB": np.ascontiguousarray(xc[SPLIT:]).astype(_NP_BF16),
        "iota": np.tile(np.arange(128, dtype=np.float32), (128, 1)).astype(_NP_BF16),
    }
    sched = (tuple(int(t) for t in tAmax), tuple(int(t) for t in tBmax))
    return in_maps, shared, sched


# ----------------------------------------------------------------------------
# Kernel builder
# ----------------------------------------------------------------------------

def _build(sched):
    tAmax, tBmax = np.array(sched[0]), np.array(sched[1])
    offA = np.concatenate([[0], np.cumsum(tAmax)])
    offB = np.concatenate([[0], np.cumsum(tBmax)])
    totA, totB = int(offA[-1]), int(offB[-1])
    callsA, callsB = -(-totA // NT), -(-totB // NT)
    padA, padB = callsA * NT, callsB * NT
    CB = N_CLIQUES - SPLIT

    from concourse.bacc import Bacc

    nc = Bacc(None, num_swdge_queues=NQ)
    xcA = nc.declare_dram_parameter("xcA", [SPLIT, D], _BF16, isOutput=False)
    xcB = nc.declare_dram_parameter("xcB", [CB, D], _BF16, isOutput=False)
    idxA = nc.declare_dram_parameter("idxA", [128, padA * 8], mybir.dt.int16, isOutput=False)
    idxB = nc.declare_dram_parameter("idxB", [128, padB * 8], mybir.dt.int16, isOutput=False)
    destA = nc.declare_dram_parameter("destA", [128, padA], _BF16, isOutput=False)
    destB = nc.declare_dram_parameter("destB", [128, padB], _BF16, isOutput=False)
    invc = nc.declare_dram_parameter("invc", [128, NBLK], _F32, isOutput=False)
    iota = nc.declare_dram_parameter("iota", [128, 128], _BF16, isOutput=False)
    wt = nc.declare_dram_parameter("wt", [128, 128], _BF16, isOutput=False)
    brow = nc.declare_dram_parameter("brow", [1, 128], _BF16, isOutput=False)
    cntb = nc.declare_dram_parameter("cntb", [1, NPAD], _BF16, isOutput=False)
    out = nc.declare_dram_parameter("out", [NPAD, D], _F32, isOutput=True)

    # Gather-call lists per stream: full NT-tile calls, but the final call is
    # split into <=8-tile sub-calls so the tail blocks' data lands sooner.
    def call_list(tot):
        calls = []
        t = 0
        while t < tot:
            nt = NT if tot - t > NT else min(8, tot - t)
            calls.append((t, nt))
            t += nt
        return calls

    callsA_l = call_list(totA)
    callsB_l = call_list(totB)

    # merged gather-call order: by first block each call serves (A first on tie)
    def start_block(off, t0):
        return int(np.searchsorted(off, t0, side="right") - 1)

    merged = sorted(
        [(start_block(offA, t0), 0, i) for i, (t0, nt) in enumerate(callsA_l)]
        + [(start_block(offB, t0), 1, i) for i, (t0, nt) in enumerate(callsB_l)],
        key=lambda t: (t[0], t[1], t[2]),
    )

    # tile index -> (call index, slot) per stream
    def tile_map(calls):
        m = {}
        for i, (t0, nt) in enumerate(calls):
            for s in range(nt):
                m[t0 + s] = (i, s)
        return m

    tmapA = tile_map(callsA_l)
    tmapB = tile_map(callsB_l)

    from contextlib import ExitStack

    with PatchedTileContext(nc) as tc, ExitStack() as ctx:
        const = ctx.enter_context(tc.tile_pool(name="const", bufs=1))
        gpool = ctx.enter_context(tc.tile_pool(name="g", bufs=3))
        opool = ctx.enter_context(tc.tile_pool(name="o", bufs=3))
        sb = ctx.enter_context(tc.tile_pool(name="sb", bufs=2))
        ps = ctx.enter_context(tc.tile_pool(name="ps", bufs=4, space="PSUM"))
        psl = ctx.enter_context(tc.tile_pool(name="psl", bufs=2, space="PSUM"))

        # idx consts loaded in chunks so the first gathers wait only on the
        # slices they read, not the whole index upload.
        idxA_t = const.tile([128, padA * 8], mybir.dt.int16)
        h1 = (padA * 8) // 4
        nc.sync.dma_start(idxA_t[:, :h1], idxA[:, :h1])
        nc.sync.dma_start(idxA_t[:, h1:], idxA[:, h1:])
        idxB_t = const.tile([128, padB * 8], mybir.dt.int16)
        h2 = (padB * 8) // 4
        nc.sync.dma_start(idxB_t[:, :h2], idxB[:, :h2])
        nc.sync.dma_start(idxB_t[:, h2:], idxB[:, h2:])
        destA_t = const.tile([128, padA], _BF16)
        nc.sync.dma_start(destA_t[:], destA[:])
        destB_t = const.tile([128, padB], _BF16)
        nc.sync.dma_start(destB_t[:], destB[:])
        invc_t = const.tile([128, NBLK], _F32)
        nc.sync.dma_start(invc_t[:], invc[:])
        iota_t = const.tile([128, 128], _BF16)
        nc.sync.dma_start(iota_t[:], iota[:])
        wt_t = const.tile([128, 128], _BF16)
        nc.sync.dma_start(wt_t[:], wt[:])
        brow_t = const.tile([1, 128], _BF16)
        nc.sync.dma_start(brow_t[:], brow[:])
        cntb_t = const.tile([1, NPAD], _BF16)
        nc.sync.dma_start(cntb_t[:], cntb[:])

        call_tiles = {}   # (stream, k) -> (gathered tile, onehot tile)
        emitted = [0]

        def emit_calls(up_to_block):
            while emitted[0] < len(merged) and merged[emitted[0]][0] <= up_to_block:
                _, stream, k = merged[emitted[0]]
                qi = emitted[0] % NQ
                src = xcA if stream == 0 else xcB
                idx_t = idxA_t if stream == 0 else idxB_t
                dest_t = destA_t if stream == 0 else destB_t
                t0, nt = (callsA_l if stream == 0 else callsB_l)[k]
                g_t = gpool.tile([128, NT, 128], _BF16, tag=f"g{qi}")
                nc.gpsimd.dma_gather(
                    g_t[:, :nt, :],
                    src[:],
                    idx_t[:, t0 * 8 : (t0 + nt) * 8],
                    nt * 128,
                    nt * 128,
                    D,
                    single_packet=False,
                    queue_num=qi,
                )
                oh_t = opool.tile([128, NT, 128], _BF16, tag=f"o{qi}")
                nc.vector.tensor_tensor(
                    out=oh_t[:, :nt, :],
                    in0=dest_t[:, t0 : t0 + nt, None].to_broadcast(
                        [128, nt, 128]
                    ),
                    in1=iota_t[:, None, :].to_broadcast([128, nt, 128]),
                    op=mybir.AluOpType.is_equal,
                )
                call_tiles[(stream, k)] = (g_t, oh_t)
                emitted[0] += 1

        for b in range(NBLK):
            emit_calls(b)
            mms = [(0, t) for t in range(int(offA[b]), int(offA[b + 1]))] + [
                (1, t) for t in range(int(offB[b]), int(offB[b + 1]))
            ]
            accum = ps.tile([128, 128], _F32, tag="acc")
            for i, (stream, t) in enumerate(mms):
                ci, slot = (tmapA if stream == 0 else tmapB)[t]
                g_t, oh_t = call_tiles[(stream, ci)]
                nc.tensor.matmul(
                    out=accum[:],
                    lhsT=g_t[:, slot, :],
                    rhs=oh_t[:, slot, :],
                    start=(i == 0),
                    stop=(i == len(mms) - 1),
                )
            # accum[f, n] is summed.T — exactly the lhsT the Linear wants.
            acc_sb = sb.tile([128, 128], _BF16, tag="accsb")
            nc.scalar.activation(
                acc_sb[:], accum[:], mybir.ActivationFunctionType.Copy
            )
            # lin[n, o] = summed[n, :] @ W.T + max(cnt[n],1)*b[o]; the rank-1
            # count*bias term makes the later 1/max(cnt,1) scale yield "+b".
            lin = psl.tile([128, 128], _F32, tag="lin")
            nc.tensor.matmul(
                out=lin[:], lhsT=acc_sb[:], rhs=wt_t[:], start=True, stop=False
            )
            nc.tensor.matmul(
                out=lin[:],
                lhsT=cntb_t[:, b * 128 : (b + 1) * 128],
                rhs=brow_t[:],
                start=False,
                stop=True,
            )
            # out[n, o] = lin[n, o] / max(count[n], 1)
            sc = sb.tile([128, 128], _F32, tag="sc")
            nc.scalar.activation(
                sc[:],
                lin[:],
                mybir.ActivationFunctionType.Copy,
                scale=invc_t[:, b : b + 1],
            )
            nc.sync.dma_start(out[b * 128 : (b + 1) * 128, :], sc[:])

    nc.finalize()
    return nc


_BUILD_CACHE = {}


def kernel(x, x_clique, node2clique_index, W, b, _trace=False, _tmpdir=None):
    in_maps, shared, sched = _prepare(x_clique, node2clique_index)

    shared["wt"] = np.ascontiguousarray(
        np.asarray(W, dtype=np.float32).T
    ).astype(_NP_BF16)
    shared["brow"] = np.asarray(b, dtype=np.float32)[None, :].astype(_NP_BF16)

    if sched not in _BUILD_CACHE:
        _BUILD_CACHE[sched] = _build(sched)
    nc = _BUILD_CACHE[sched]

    full_maps = [dict(m, **shared) for m in in_maps]
    kwargs = {}
    if _trace:
        kwargs = dict(trace=True, tmpdir=_tmpdir)
    res = run_bass_kernel_spmd(nc, full_maps, core_ids=list(range(N_CORES)), **kwargs)

    out = np.concatenate(
        [res.results[c]["out"][:NPC] for c in range(N_CORES)], axis=0
    ).astype(np.float32)
    if _trace:
        return out, res
    return out
